# revision 1
# baseline (speedup 1.0000x reference)
"""Causal multi-head attention block, sharded over 8 TRN2 NeuronCores.

Sharding: core c handles batch b = c//2 and head-group g = c%2 (8 of 16 heads).
Each core computes QKV projections, causal flash-style attention, and a
partial output projection for its head group; the host sums the two
head-group partials per batch (partial-sum unshard) and adds b_O.

On-device layouts (per core, S=2048, M=1024, H8=8 heads, Dh=64):
  x_t     [1024, 2048]  x[b] transposed (host pre-transposes)     bf16
  QT/KT   4 pair-tiles [128, 2048]: partition = (head-in-pair, d) bf16
  Vones   16 s-tiles [128, 520]: row=key pos, col=65*h+d, d=64 → 1.0 bf16
  zT_all  4 pair-tiles [128, 2048] (normalized z^T)               bf16
  out_t   [1024, 2048]  partial (out proj)^T, host sums + transposes
Scores are computed transposed (S^T[key, query]) so softmax denominators
come from an extra all-ones column in V (matmul partition reduction), and
no on-chip transposes are needed anywhere.

Perf structure (v2):
 - ~12 warm-up matmuls on a memset tile heat the PE HAM clock-gate toward
   2.4 GHz while the first DMAs stream in.
 - x_t loads seq-major and q/k weights per-pair, so the v_proj / qk_proj /
   attention pipeline starts as soon as the first chunks land; pair 0 is
   interleaved with v_proj.
 - One shared PSUM ring (4x [128,512] banks) serves v/qk/z/bc/out-proj
   tiles; scores keep a 2x [128,1024] double buffer: exactly 8 banks.
 - Normalization is pipelined, no DRAM roundtrip: z and den rows are
   staged to SBUF right after each z-accumulation (freeing the PSUM bank),
   dens collect into per-(pair,head) [1,2048] tiles, the reciprocals run
   as two batched ACT ops in the ACT-idle qk_proj window at the pair
   boundary (one table-set switch each way), and the 1/den broadcast
   (small matmul) + in-place z multiply flush into the next pair's
   attention stream.
 - Pair 3 splits its reciprocal batch in half so out-proj for the first
   two query superblocks hides inside its last two attention blocks.
"""

import sys

if "/opt/trn_rl_repo" not in sys.path:
    sys.path.insert(0, "/opt/trn_rl_repo")

import numpy as np
import ml_dtypes

import concourse.bass as bass
import concourse.mybir as mybir
from concourse import tile

BF16 = mybir.dt.bfloat16
F32 = mybir.dt.float32

B, S, M, H, DH = 4, 2048, 1024, 16, 64
H8 = 8          # heads per core
NP = 4          # head pairs per core
SB = 512        # query superblock
KB = 128        # key block
NSB = S // SB   # 4
NKB = S // KB   # 16
MK = M // 128   # 8 contraction chunks
ATTN_SCALE = 1.0 / np.sqrt(DH)
N_WARMUP = 12   # PE warm-up matmuls (HAM un-throttle needs ~3.4us busy)

# ---------------------------------------------------------------------------
# Patch: this walrus build rejects >1 sync-wait per engine instruction.
# Post-pass: for any non-DMA instruction with N>1 waits, insert N-1
# single-wait NoOps on the same engine immediately before it.
MAX_ENGINE_WAITS = 1


def split_multi_waits(nc: bass.Bass):
    n_split = 0
    for f in nc.m.functions:
        for blk in f.blocks:
            new_list = []
            for inst in blk.instructions:
                si = getattr(inst, "sync_info", None)
                waits = list(si.on_wait) if si is not None else []
                if len(waits) > MAX_ENGINE_WAITS:
                    extra = waits[: -MAX_ENGINE_WAITS]
                    keep = waits[-MAX_ENGINE_WAITS:]
                    for i in range(0, len(extra), MAX_ENGINE_WAITS):
                        nop = mybir.InstNoOp(
                            name=f"I-wsplit-{nc.next_id()}", ins=[], outs=[]
                        )
                        nop.engine = inst.engine
                        nop.sync_info = mybir.SyncInfo(
                            on_wait=extra[i : i + MAX_ENGINE_WAITS], on_update=[]
                        )
                        new_list.append(nop)
                    inst.sync_info = mybir.SyncInfo(
                        on_wait=keep, on_update=list(si.on_update)
                    )
                    n_split += 1
                new_list.append(inst)
            blk.instructions = new_list
    return n_split


def act_reciprocal(nc, out_ap, in_ap):
    """ACT-engine elementwise 1/x via raw InstActivation (accuracy ~1e-3,
    fine for softmax denominators; DVE reciprocal is 5x slower per lane)."""
    eng = nc.scalar
    inputs = [
        eng.lower_ap(in_ap),
        mybir.ImmediateValue(dtype=mybir.dt.float32, value=0.0),
        mybir.ImmediateValue(dtype=mybir.dt.float32, value=1.0),
        mybir.ImmediateValue(dtype=mybir.dt.float32, value=0.0),
    ]
    return eng.add_instruction(
        mybir.InstActivation(
            name=nc.get_next_instruction_name(),
            func=mybir.ActivationFunctionType.Reciprocal,
            ins=inputs,
            outs=[eng.lower_ap(out_ap)],
        )
    )


def build_nc() -> bass.Bass:
    nc = bass.Bass()

    x_t = nc.declare_dram_parameter("x_t", [M, S], BF16, isOutput=False)
    # w_q/w_k pre-tiled p-major: [128, (p, k, 128)] so each pair's weights
    # are one contiguous 0.5MB slab (loaded per pair, just in time).
    w_q = nc.declare_dram_parameter("w_q", [128, NP * MK * 128], BF16, isOutput=False)
    w_k = nc.declare_dram_parameter("w_k", [128, NP * MK * 128], BF16, isOutput=False)
    w_v = nc.declare_dram_parameter("w_v", [128, MK * 512], BF16, isOutput=False)
    w_o = nc.declare_dram_parameter("w_o", [128, NP * MK * 128], BF16, isOutput=False)
    b_q = nc.declare_dram_parameter("b_q", [NP, 128], F32, isOutput=False)
    b_k = nc.declare_dram_parameter("b_k", [NP, 128], F32, isOutput=False)
    b_v = nc.declare_dram_parameter("b_v", [1, H8 * DH], BF16, isOutput=False)
    out_t = nc.declare_dram_parameter("out_t", [M, S], BF16, isOutput=True)

    with tile.TileContext(nc) as tc:
        with (
            tc.tile_pool(name="persist", bufs=1) as persist,
            tc.tile_pool(name="wstream", bufs=1) as wpool,
        ):
            # --- resident tiles -------------------------------------------
            # x_t split into seq-chunk tiles so the pipeline can start as
            # soon as the first chunk lands (independent DMA targets)
            xt = [
                [
                    persist.tile([128, SB], BF16, tag=f"xt{k}_{c}", name=f"xt{k}_{c}")
                    for c in range(NSB)
                ]
                for k in range(MK)
            ]
            qt = [persist.tile([128, S], BF16, tag=f"qt{p}", name=f"qt{p}") for p in range(NP)]
            kt = [persist.tile([128, S], BF16, tag=f"kt{p}", name=f"kt{p}") for p in range(NP)]
            vones = [
                persist.tile([128, H8 * 65], BF16, tag=f"vones{sb}", name=f"vones{sb}")
                for sb in range(NKB)
            ]
            zt = [persist.tile([128, S], BF16, tag=f"zt{p}", name=f"zt{p}") for p in range(NP)]

            wq_all = wpool.tile([128, NP * MK * 128], BF16, tag="wq_all")
            wk_all = wpool.tile([128, NP * MK * 128], BF16, tag="wk_all")
            wv_all = wpool.tile([128, MK * 512], BF16, tag="wv_all")
            wo_all = wpool.tile([128, NP * MK * 128], BF16, tag="wo_all")
            # p-major: pair p, contraction chunk k
            wq = [
                [wq_all[:, (p * MK + k) * 128 : (p * MK + k + 1) * 128] for k in range(MK)]
                for p in range(NP)
            ]
            wk = [
                [wk_all[:, (p * MK + k) * 128 : (p * MK + k + 1) * 128] for k in range(MK)]
                for p in range(NP)
            ]
            wv = [wv_all[:, k * 512 : (k + 1) * 512] for k in range(MK)]
            wo = [
                [wo_all[:, (c * MK + k) * 128 : (c * MK + k + 1) * 128] for k in range(MK)]
                for c in range(NP)
            ]
            bq_t = persist.tile([128, NP], F32, tag="bq")
            bk_t = persist.tile([128, NP], F32, tag="bk")
            bv_t = persist.tile([1, H8 * DH], BF16, tag="bv")
            ones_col = persist.tile([1, 128], BF16, tag="ones_col")
            ones33 = persist.tile([33, 128], BF16, tag="ones33")
            warm_sb = persist.tile([128, 512], BF16, tag="warm_sb")

            # --- memsets (no DMA dependency; run at t=0) ------------------
            nc.gpsimd.memset(warm_sb[:], 1.0)
            nc.gpsimd.memset(ones_col[:], 1.0)
            nc.gpsimd.memset(ones33[:], 1.0)
            for sb in range(NKB):
                v3 = vones[sb][:].rearrange("p (h e) -> p h e", e=65)
                nc.gpsimd.memset(v3[:, :, 64:65], 1.0)

            # --- PE warm-up: heat the HAM clock gate while DMAs stream ----
            # (two alternating banks so WAW tracking doesn't serialize)
            with tc.tile_pool(name="warm_ps", bufs=2, space="PSUM") as warm_pool:
                for _ in range(N_WARMUP):
                    wps = warm_pool.tile([128, 512], F32, tag="warm_ps")
                    nc.tensor.matmul(
                        wps[:], warm_sb[:, 0:128], warm_sb[:], start=True, stop=True
                    )

            # --- DMA program ----------------------------------------------
            # sync ring: x_t seq-major (compute consumes seq-chunk by chunk),
            # then w_o (needed only at the output projection).
            for skb in range(NSB):
                ssl = slice(skb * SB, (skb + 1) * SB)
                for k in range(MK):
                    nc.sync.dma_start(
                        xt[k][skb][:], x_t[k * 128 : (k + 1) * 128, ssl]
                    )
            nc.sync.dma_start(wo_all[:], w_o[:])
            # scalar ring: v-path first (k-chunked so v_proj's accumulation
            # loop starts as soon as the first chunk lands), then per-pair
            # q/k weights.
            nc.scalar.dma_start(bv_t[:], b_v[:])
            for k in range(MK):
                ksl = slice(k * 512, (k + 1) * 512)
                nc.scalar.dma_start(wv_all[:, ksl], w_v[:, ksl])
            for p in range(NP):
                nc.scalar.dma_start(bq_t[:, p], b_q[p])
                nc.scalar.dma_start(bk_t[:, p], b_k[p])
            for p in range(NP):
                psl = slice(p * MK * 128, (p + 1) * MK * 128)
                nc.scalar.dma_start(wq_all[:, psl], w_q[:, psl])
                nc.scalar.dma_start(wk_all[:, psl], w_k[:, psl])

            # --- main fused phase -----------------------------------------
            # PSUM: sps 2x[128,1024] (4 banks) + zpool 2x (2) + fpool 2x (2)
            with (
                tc.tile_pool(name="fpool", bufs=2, space="PSUM") as fpool,
                tc.tile_pool(name="zpool", bufs=2, space="PSUM") as zpool,
                tc.tile_pool(name="s_ps", bufs=2, space="PSUM") as s_ps,
                tc.tile_pool(name="epool", bufs=10) as epool,
                tc.tile_pool(name="dpool", bufs=2) as dpool,
                tc.tile_pool(name="opool", bufs=4) as opool,
            ):
                # ---- filler stream: projection / norm / out-proj matmuls
                # dripped into the ACT-bound attention loop, ~FEED_NS of PE
                # work per key block, so the PE never starves while exp runs.
                FEED_NS = 330
                fq = []          # [key, generator]
                done_keys = set()

                def feed(ns):
                    while fq and ns > 0:
                        key, g = fq[0]
                        step = next(g, None)
                        if step is None:
                            done_keys.add(key)
                            fq.pop(0)
                        else:
                            ns -= step
                    return ns

                def drain_until(key):
                    while fq and key not in done_keys:
                        k, g = fq[0]
                        if next(g, None) is None:
                            done_keys.add(k)
                            fq.pop(0)

                def drain_all():
                    while fq:
                        k, g = fq[0]
                        if next(g, None) is None:
                            done_keys.add(k)
                            fq.pop(0)

                def gen_v(kb):
                    skb, r = divmod(kb, 4)
                    ksl = slice(r * KB, (r + 1) * KB)
                    ps_v = fpool.tile([128, 512], F32, tag="fp", name="ps_v")
                    for k in range(MK):
                        nc.tensor.matmul(
                            ps_v[:], xt[k][skb][:, ksl], wv[k],
                            start=(k == 0), stop=False,
                        )
                        yield 213
                    nc.tensor.matmul(
                        ps_v[:], ones_col[:], bv_t[:], start=False, stop=True
                    )
                    yield 213
                    v3 = vones[kb][:].rearrange("p (h e) -> p h e", e=65)
                    nc.vector.tensor_copy(
                        v3[:, :, 0:64], ps_v[:].rearrange("p (h e) -> p h e", e=64)
                    )

                def gen_qk(p, sb):
                    ssl = slice(sb * SB, (sb + 1) * SB)
                    for w_t, b_t, dst in ((wq, bq_t, qt), (wk, bk_t, kt)):
                        ps = fpool.tile([128, 512], F32, tag="fp", name="ps_qk")
                        for k in range(MK):
                            nc.tensor.matmul(
                                ps[:], w_t[p][k], xt[k][sb][:],
                                start=(k == 0), stop=(k == MK - 1),
                            )
                            yield 213
                        nc.vector.tensor_scalar_add(
                            dst[p][:, ssl], ps[:], b_t[:, p : p + 1]
                        )

                def gen_chain(p, j):
                    qsl = slice(j * SB, (j + 1) * SB)
                    for h2 in range(2):
                        rows = slice(h2 * 64, h2 * 64 + 64)
                        r0 = 32 * h2
                        bc = fpool.tile([128, 512], F32, tag="fp", name="bc")
                        nc.tensor.matmul(
                            bc[0:64, :],
                            ones33[r0 : r0 + 1, 0:64],
                            rcp_t[p][r0 : r0 + 1, qsl],
                            start=True,
                            stop=True,
                        )
                        yield 213
                        nc.vector.tensor_tensor(
                            zt[p][rows, qsl],
                            zt[p][rows, qsl],
                            bc[0:64, :],
                            op=mybir.AluOpType.mult,
                        )

                def gen_out(j):
                    qsl = slice(j * SB, (j + 1) * SB)
                    for k in range(MK):
                        ps_o = fpool.tile([128, 512], F32, tag="fp", name="ps_o")
                        for c in range(NP):
                            nc.tensor.matmul(
                                ps_o[:], wo[c][k], zt[c][:, qsl],
                                start=(c == 0), stop=(c == NP - 1),
                            )
                            yield 213
                        ot = opool.tile([128, SB], BF16, tag="ot", name="ot")
                        nc.vector.tensor_copy(ot[:], ps_o[:])
                        nc.sync.dma_start(
                            out_t[k * 128 : (k + 1) * 128, qsl], ot[:]
                        )

                def enq(key, g):
                    fq.append((key, g))

                # den / rcp staging: per pair, [33, 2048] with head h2 on
                # partition row 32*h2 (rows {0,32} are valid bc-matmul
                # tile_position rows, and one strided-partition ACT op
                # covers both heads at free-size cost)
                den_t = {}
                rcp_t = {}

                def get_den(p, h2):
                    if p not in den_t:
                        den_t[p] = dpool.tile(
                            [33, S], F32, tag="den", name=f"den{p}", bufs=2
                        )
                        rcp_t[p] = dpool.tile(
                            [33, S], BF16, tag="rcp", name=f"rcp{p}", bufs=2
                        )
                    return den_t[p][32 * h2 : 32 * h2 + 1, :]

                def recip_batch(p, qsl=slice(0, S), engine="act"):
                    # 1/den. ACT path: exp(-ln(den)) -- both functions live
                    # in the natural_log_exp_and_others table set, the same
                    # set the attention exps use, so NO table switching;
                    # one strided op covers both heads.
                    # DVE path (for slices mid-attention): native reciprocal
                    # + bf16 cast, keeping the exp stream on ACT undisturbed.
                    get_den(p, 0)
                    if engine == "act":
                        for h2 in range(2):
                            r0 = 32 * h2
                            nc.scalar.activation(
                                den_t[p][r0 : r0 + 1, qsl],
                                den_t[p][r0 : r0 + 1, qsl],
                                mybir.ActivationFunctionType.Ln,
                            )
                            nc.scalar.activation(
                                rcp_t[p][r0 : r0 + 1, qsl],
                                den_t[p][r0 : r0 + 1, qsl],
                                mybir.ActivationFunctionType.Exp,
                                scale=-1.0,
                            )
                    else:
                        for h2 in range(2):
                            r0 = 32 * h2
                            scr = dpool.tile(
                                [1, qsl.stop - qsl.start], F32, tag="rscr",
                                name="rscr", bufs=2,
                            )
                            nc.vector.reciprocal(scr[:], den_t[p][r0 : r0 + 1, qsl])
                            nc.vector.tensor_copy(
                                rcp_t[p][r0 : r0 + 1, qsl], scr[:]
                            )

                def attention(p, j):
                    qsl = slice(j * SB, (j + 1) * SB)
                    nk = 4 * (j + 1)
                    zps = [
                        zpool.tile([128, 512], F32, tag="zp", name="zps0"),
                        zpool.tile([128, 512], F32, tag="zp", name="zps1"),
                    ]

                    def z_mms(kbz, e_tile):
                        # crossing blocks: queries < 128i are fully masked
                        i = kbz - 4 * j
                        c0 = 128 * i if i > 0 else 0
                        for h2 in range(2):
                            h = 2 * p + h2
                            nc.tensor.matmul(
                                zps[h2][0:65, c0:],
                                vones[kbz][:, 65 * h : 65 * h + 65],
                                e_tile[:, h2 * SB + c0 : (h2 + 1) * SB],
                                start=(kbz == 0),
                                stop=(kbz == nk - 1),
                            )

                    pend = []
                    for kb in range(nk):
                        i = kb - 4 * j
                        c0 = 128 * i if i > 0 else 0
                        if kb >= 2:
                            # filler BEFORE this block's matmuls: the PE
                            # works these while ACT catches up on exp(kb-2),
                            # whose sps bank the next scores matmul waits on.
                            # Budget = this block's ACT-vs-PE deficit.
                            cols = 2 * (SB - c0)
                            exp_ns = cols / 1.2 + 293
                            pe_ns = 1.5 * cols * 0.4167 + 160
                            if feed(max(0.0, exp_ns - pe_ns)) > 0:
                                # queue dry: tiny dummy matmul into the unused
                                # partitions of the live zps bank keeps the
                                # HAM activity monitor from re-throttling the
                                # PE clock to 1.2 GHz during ACT-paced blocks
                                nc.tensor.matmul(
                                    zps[kb % 2][96:128, 0:64],
                                    warm_sb[:, 0:32],
                                    warm_sb[:, 0:64],
                                    start=False,
                                    stop=False,
                                    tile_position=(0, 96),
                                    skip_group_check=True,
                                )
                        ksl = slice(kb * KB, (kb + 1) * KB)
                        sps = s_ps.tile([128, 2 * SB], F32, tag="sps", name="sps")
                        for h2 in range(2):
                            rows = slice(h2 * 64, h2 * 64 + 64)
                            nc.tensor.matmul(
                                sps[:, h2 * SB + c0 : (h2 + 1) * SB],
                                kt[p][rows, ksl],
                                qt[p][rows, j * SB + c0 : (j + 1) * SB],
                            )
                        e = epool.tile([128, 2 * SB], BF16, tag="e", name="e")
                        if c0:
                            e3 = e[:].rearrange("p (h q) -> p h q", q=SB)
                            s3 = sps[:].rearrange("p (h q) -> p h q", q=SB)
                            nc.scalar.activation(
                                e3[:, :, c0:],
                                s3[:, :, c0:],
                                mybir.ActivationFunctionType.Exp,
                                scale=float(ATTN_SCALE),
                            )
                        else:
                            nc.scalar.activation(
                                e[:],
                                sps[:],
                                mybir.ActivationFunctionType.Exp,
                                scale=float(ATTN_SCALE),
                            )
                        if i >= 0:
                            # zero the strictly-upper part of the diagonal
                            # 128-wide stripe
                            e3 = e[:].rearrange("p (h q) -> p h q", q=SB)
                            nc.gpsimd.affine_select(
                                out=e3[:, :, c0 : c0 + 128],
                                in_=e3[:, :, c0 : c0 + 128],
                                compare_op=mybir.AluOpType.is_ge,
                                fill=0.0,
                                base=j * SB - kb * KB + c0,
                                pattern=[[0, 2], [1, 128]],
                                channel_multiplier=-1,
                            )
                        pend.append((kb, e))
                        if len(pend) > 2:
                            z_mms(*pend.pop(0))
                    for it in pend:
                        feed(2 * FEED_NS)
                        z_mms(*it)

                    # stage z and den to SBUF right away (frees the banks)
                    for h2 in range(2):
                        rows = slice(h2 * 64, h2 * 64 + 64)
                        nc.vector.tensor_copy(zt[p][rows, qsl], zps[h2][0:64, :])
                        nc.vector.tensor_copy(
                            get_den(p, h2)[0:1, qsl], zps[h2][64:65, :]
                        )

                # --- pipelined emission ----------------------------------
                # pair 0: v/qk dense (pair 0 is PE-rich; DMA-paced anyway)
                for kb in range(4):
                    enq(("v", kb), gen_v(kb))
                enq(("qk", 0, 0), gen_qk(0, 0))
                for j in range(NSB):
                    drain_until(("v", 4 * j + 3))
                    drain_until(("qk", 0, j))
                    if j < 3:
                        for kb in range(4 * j + 4, 4 * j + 8):
                            enq(("v", kb), gen_v(kb))
                        enq(("qk", 0, j + 1), gen_qk(0, j + 1))
                    else:
                        # only the later superblocks become fillers; qk(1,0/1)
                        # are held back as dense PE work for the recip burst
                        # at the pair boundary
                        enq(("qk", 1, 2), gen_qk(1, 2))
                        enq(("qk", 1, 3), gen_qk(1, 3))
                    attention(0, j)
                for p in range(1, NP):
                    last = p == NP - 1
                    # pair boundary: the held-back qk(p,0/1) run dense on
                    # the PE while ACT does the previous pair's reciprocals;
                    # chains enqueue after att(p,0) (no head-of-line block)
                    enq(("qk", p, 0), gen_qk(p, 0))
                    enq(("qk", p, 1), gen_qk(p, 1))
                    recip_batch(p - 1)
                    if not last:
                        enq(("qk", p + 1, 2), gen_qk(p + 1, 2))
                        enq(("qk", p + 1, 3), gen_qk(p + 1, 3))
                    for j in range(NSB):
                        if j == 2:
                            # two full attentions of ACT queue have passed:
                            # the reciprocals are surely done, chains can't
                            # head-of-line-block the PE
                            for jc in range(NSB):
                                enq(("chain", p - 1, jc), gen_chain(p - 1, jc))
                        drain_until(("qk", p, j))
                        if last and j == 2:
                            # pair-3 split: first-half norms + out-proj(0,1)
                            # become fillers for its last two attentions
                            recip_batch(p, qsl=slice(0, 2 * SB))
                            enq(("chain", p, 0), gen_chain(p, 0))
                            enq(("chain", p, 1), gen_chain(p, 1))
                            enq(("out", 0), gen_out(0))
                            enq(("out", 1), gen_out(1))
                        attention(p, j)
                        if last and j == 2:
                            # j2 reciprocal on DVE (ACT keeps streaming
                            # att(3,3)'s exps); out-proj(2) joins the fillers
                            recip_batch(
                                p, qsl=slice(2 * SB, 3 * SB), engine="dve"
                            )
                            enq(("chain", p, 2), gen_chain(p, 2))
                            enq(("out", 2), gen_out(2))
                # tail: remaining norm + out-proj, PE kept dense and warm
                drain_all()
                recip_batch(NP - 1, qsl=slice(3 * SB, S))
                enq(("chain", NP - 1, 3), gen_chain(NP - 1, 3))
                enq(("out", 3), gen_out(3))
                drain_all()

    split_multi_waits(nc)
    return nc


_CACHED = {}


def _get_nc():
    if "nc" not in _CACHED:
        _CACHED["nc"] = build_nc()
    return _CACHED["nc"]


def kernel(
    x,
    pos_embed,
    W_Q,
    b_Q,
    W_K,
    b_K,
    W_V,
    b_V,
    W_O,
    b_O,
    _want_results=False,
    _trace=False,
):
    from concourse.bass_utils import run_bass_kernel_spmd

    bf16 = ml_dtypes.bfloat16
    x = np.asarray(x, np.float32)
    W_Q = np.asarray(W_Q, np.float32)
    b_Q = np.asarray(b_Q, np.float32)
    W_K = np.asarray(W_K, np.float32)
    b_K = np.asarray(b_K, np.float32)
    W_V = np.asarray(W_V, np.float32)
    b_V = np.asarray(b_V, np.float32)
    W_O = np.asarray(W_O, np.float32)
    b_O = np.asarray(b_O, np.float32)

    in_maps = []
    for c in range(8):
        b, g = divmod(c, 2)
        hs = slice(g * H8, (g + 1) * H8)
        # [H8, M, DH] -> [M, H8*DH] with col = 64*h + d, then pre-tiled into
        # the on-chip layout: [128 part, (p, k, 128)] for Q/K (p-major),
        # [128, (k, 512)] for V, [128, (c, k, 128)] for O.
        wq_f = W_Q[hs].transpose(1, 0, 2).reshape(M, H8 * DH)
        wk_f = W_K[hs].transpose(1, 0, 2).reshape(M, H8 * DH)
        wv_f = W_V[hs].transpose(1, 0, 2).reshape(M, H8 * DH)
        wo_f = W_O[hs].reshape(H8 * DH, M)
        # [M=MK*128, NP*128] -> [MK, 128, NP, 128] -> [128, NP, MK, 128]
        wq = np.ascontiguousarray(
            wq_f.reshape(MK, 128, NP, 128).transpose(1, 2, 0, 3).reshape(128, -1)
        )
        wk = np.ascontiguousarray(
            wk_f.reshape(MK, 128, NP, 128).transpose(1, 2, 0, 3).reshape(128, -1)
        )
        wv = np.ascontiguousarray(
            wv_f.reshape(MK, 128, 512).transpose(1, 0, 2).reshape(128, -1)
        )
        # [H8*DH=NP*128, M=MK*128] -> [NP, 128, MK, 128] -> [128, NP, MK, 128]
        wo = np.ascontiguousarray(
            wo_f.reshape(NP, 128, MK, 128).transpose(1, 0, 2, 3).reshape(128, -1)
        )
        in_maps.append(
            {
                "x_t": np.ascontiguousarray(x[b].T).astype(bf16),
                "w_q": wq.astype(bf16),
                "w_k": wk.astype(bf16),
                "w_v": wv.astype(bf16),
                "w_o": wo.astype(bf16),
                "b_q": np.ascontiguousarray(b_Q[hs].reshape(NP, 128)),
                "b_k": np.ascontiguousarray(b_K[hs].reshape(NP, 128)),
                "b_v": b_V[hs].reshape(1, H8 * DH).astype(bf16),
            }
        )

    nc = _get_nc()
    res = run_bass_kernel_spmd(nc, in_maps, list(range(8)), trace=_trace)

    out = np.empty((B, S, M), np.float32)
    for b in range(B):
        p0 = res.results[2 * b]["out_t"].astype(np.float32)
        p1 = res.results[2 * b + 1]["out_t"].astype(np.float32)
        out[b] = (p0 + p1).T + b_O
    if _want_results:
        return out, res
    return out

